# revision 1
# baseline (speedup 1.0000x reference)
"""BiGRU encoder (nn_BiGRUEncoder) as an 8-core TRN2 Bass kernel.

Contract: kernel(**inputs) takes the FULL unsharded inputs from
setup_inputs() and returns the FULL [B, T-2L, 2F] output, distributing work
across 8 NeuronCores internally.

Decomposition: the hidden dim F=1024 is split across the 8 cores (128
features each). Every core runs BOTH scan directions with the full batch
B=32, computing its 384 rows of the 3F gate pre-activations per step. After
each step the transposed h chunks ([128, 32] per direction) are exchanged
with an AllGather so the next step's recurrent matmul has the full h.T.
Input projections gi = x @ Wih.T don't depend on h and are hoisted into a
prologue as one large batched matmul per direction, stored in DRAM, and
streamed per step.

Per-step layouts: batch on partitions for gate math, with both directions
stacked ([64, X]: fwd rows 0-31, bwd rows 32-63); features on partitions for
the exchanged h.T chunks. The scan stops at T-L: the last L steps of either
direction feed no output.
"""

import sys

sys.path.insert(0, "/opt/trn_rl_repo")

import os

import numpy as np

from concourse import bass, bacc, tile, mybir
from concourse import bass_utils

F32 = mybir.dt.float32

B = 32  # batch
T = 512  # sequence length
F = 1024  # hidden/feature dim
L = 10  # trim at both ends of T
NC = 8  # cores
P = 128  # partitions / features per core
G = 3 * P  # gate rows per core
KB = F // P  # contraction blocks


def build_gru_kernel(nc, tc, with_gbias: bool, with_nbias: bool):
    """Emit the SPMD program (identical on all 8 cores)."""
    ablate = os.environ.get("K_ABLATE", "")
    TS = 1 if ablate == "prologue" else T - L  # scan steps needed
    TO = T - 2 * L  # output steps

    TB8 = T * B // NC
    xt = nc.dram_tensor("xt", [F, TB8], F32, kind="ExternalInput").ap()
    wih = nc.dram_tensor("wih", [2, KB, P, G], F32, kind="ExternalInput").ap()
    whh = nc.dram_tensor("whh", [2, KB, P, G], F32, kind="ExternalInput").ap()
    ident = nc.dram_tensor("ident", [2 * B, 2 * B], F32, kind="ExternalInput").ap()
    identP = nc.dram_tensor("identP", [P, P], F32, kind="ExternalInput").ap()
    if with_gbias:
        gbias = nc.dram_tensor("gbias", [2, P, G], F32, kind="ExternalInput").ap()
    if with_nbias:
        nbias = nc.dram_tensor("nbias", [2 * B, P], F32, kind="ExternalInput").ap()
    outp = nc.dram_tensor("out_own", [2, TO, B, P], F32, kind="ExternalOutput").ap()

    whh_sb = nc.alloc_sbuf_tensor("whh_sb", [P, 2 * KB * G], F32)
    hbuf = nc.alloc_sbuf_tensor("hbuf", [2 * B, 8 * P], F32)
    ident_sb = nc.alloc_sbuf_tensor("ident_sb", [2 * B, 2 * B], F32)
    identP_sb = nc.alloc_sbuf_tensor("identP_sb", [P, P], F32)
    if with_gbias:
        gbias_sb = nc.alloc_sbuf_tensor("gbias_sb", [P, 2 * G], F32)
    if with_nbias:
        nbias_sb = nc.alloc_sbuf_tensor("nbias_sb", [2 * B, P], F32)

    if True:
        # ================= prologue =================
        for d in (0, 1):
            for k in range(KB):
                off = (d * KB + k) * G
                nc.sync.dma_start(whh_sb.ap()[:, off : off + G], whh[d, k])
        nc.sync.dma_start(ident_sb.ap(), ident)
        nc.sync.dma_start(identP_sb.ap(), identP)
        if with_gbias:
            for d in (0, 1):
                nc.sync.dma_start(gbias_sb.ap()[:, d * G : (d + 1) * G], gbias[d])
        if with_nbias:
            nc.sync.dma_start(nbias_sb.ap(), nbias)
        nc.vector.memset(hbuf.ap(), 0.0)

        # Bulk input projections, T-sliced: this core computes gi for ALL
        # cores' gate columns over its own T/8 slice, then an AllToAll gives
        # every core its own 384 columns for all T. Wih is shipped own-cols
        # and AllGathered to full on device (cuts H2D 8x).
        pidv = nc.sync.partition_id()
        with tc.tile_pool(name="wag", bufs=1, space="DRAM") as wag:
            wihf = [
                wag.tile([NC * KB * P, G], F32, name=f"wihf{d}", addr_space="Shared")
                for d in (0, 1)
            ]
            win = wag.tile([KB * P, G], F32, name="win")
            for d in (0, 1):
                nc.sync.dma_start(
                    win[:], wih[d].rearrange("k p g -> (k p) g")
                )
                nc.gpsimd.collective_compute(
                    "AllGather",
                    mybir.AluOpType.bypass,
                    replica_groups=[list(range(NC))],
                    ins=[win.opt()],
                    outs=[wihf[d].opt()],
                )
            # wihf[d] rows: (src_core r, k, p) -> Wih_d.T[128k:128k+128, r's 384]
            a2a_in = [
                wag.tile([NC * TB8, G + P], F32, name=f"a2ain{d}")
                for d in (0, 1)
            ]
            a2a_out = [
                wag.tile([NC * TB8, G + P], F32, name=f"a2aout{d}")
                for d in (0, 1)
            ]
            n_m = TB8 // P  # 16 m-tiles over this core's T-slice
            with (
                tc.tile_pool(name="xtp", bufs=3) as xtp,
                tc.tile_pool(name="wfp", bufs=1) as wfp,
                tc.tile_pool(name="gps", bufs=4, space="PSUM") as gps,
                tc.tile_pool(name="gis", bufs=4) as gis,
                tc.tile_pool(name="tpp", bufs=2, space="PSUM") as tpp,
                tc.tile_pool(name="xos", bufs=3) as xos,
            ):
                for d in (0, 1):
                    # full Wih for this direction, SBUF-resident once
                    wfull = wfp.tile([P, NC * KB * G], F32, tag="wfull")
                    nc.sync.dma_start(
                        wfull[:].rearrange("p (r k g) -> p r k g", r=NC, k=KB),
                        wihf[d][:].rearrange("(r k p) g -> p r k g", p=P, k=KB),
                    )
                    for m in range(n_m):
                        xtile = xtp.tile([P, KB * P], F32)
                        nc.sync.dma_start(
                            xtile[:].rearrange("p (k m) -> p k m", k=KB),
                            xt.rearrange("(k p) n -> p k n", p=P)[
                                :, :, m * P : (m + 1) * P
                            ],
                        )
                        if d == 0:
                            # x.T blocks for the residual: all 8 f-chunks
                            for r in range(NC):
                                xps = tpp.tile([P, P], F32)
                                nc.tensor.transpose(
                                    xps[:],
                                    xtile[:, P * r : P * (r + 1)],
                                    identP_sb.ap(),
                                )
                                xsb = xos.tile([P, P], F32, tag="xsb")
                                nc.scalar.copy(xsb[:], xps[:])
                                for dd in (0, 1):
                                    nc.sync.dma_start(
                                        a2a_in[dd][
                                            r * TB8 + m * P : r * TB8 + (m + 1) * P,
                                            G : G + P,
                                        ],
                                        xsb[:],
                                    )
                        for r in range(NC):
                            ps = gps.tile([P, G], F32)
                            for k in range(KB):
                                nc.tensor.matmul(
                                    ps[:],
                                    xtile[:, P * k : P * (k + 1)],
                                    wfull[:, (r * KB + k) * G : (r * KB + k + 1) * G],
                                    start=(k == 0),
                                    stop=(k == KB - 1),
                                )
                            gt = gis.tile([P, G], F32)
                            if with_gbias:
                                nc.vector.tensor_add(
                                    gt[:],
                                    ps[:],
                                    gbias_sb.ap()[:, d * G : (d + 1) * G],
                                )
                            else:
                                nc.scalar.copy(gt[:], ps[:])
                            nc.sync.dma_start(
                                a2a_in[d][
                                    r * TB8 + m * P : r * TB8 + (m + 1) * P, :G
                                ],
                                gt[:],
                            )
            for d in (0, 1):
                nc.gpsimd.collective_compute(
                    "AllToAll",
                    mybir.AluOpType.bypass,
                    replica_groups=[list(range(NC))],
                    ins=[a2a_in[d].opt()],
                    outs=[a2a_out[d].opt()],
                )
            # after A2A, shard s of a2a_out[d] holds rows for t in
            # [s*T/8, (s+1)*T/8) x B, own 384 cols (+x for d=0) -> global
            # t-major order, i.e. exactly gid[d].
            gid = a2a_out

        # ================= scan =================
        with (
            tc.tile_pool(name="gip", bufs=6) as gip,
            tc.tile_pool(name="srz", bufs=3) as srzp,
            tc.tile_pool(name="rzp", bufs=3) as rzp,
            tc.tile_pool(name="sml", bufs=3) as sml,
            tc.tile_pool(name="snd", bufs=3) as sndp,
            tc.tile_pool(name="gth", bufs=3) as gthp,
            tc.tile_pool(name="cin", bufs=3, space="DRAM") as cinp,
            tc.tile_pool(name="cout", bufs=3, space="DRAM") as coutp,
            tc.tile_pool(name="pmm", bufs=3, space="PSUM") as pmm,
            tc.tile_pool(name="ptr", bufs=2, space="PSUM") as ptr,
        ):
            gth_prev = None
            for t in range(TS):
                gi_t = gip.tile([2 * B, G + P], F32)
                nc.sync.dma_start(
                    gi_t[:B, :], gid[0][t * B : (t + 1) * B, :]
                )
                idx = T - 1 - t
                nc.sync.dma_start(
                    gi_t[B:, :], gid[1][idx * B : (idx + 1) * B, :]
                )
                xo_t = gi_t[:, G : G + P]

                sl = t % 8
                if t == 0:
                    # h(-1) = 0 -> gh = 0: h = (1-z)*n + x
                    zc = sml.tile([2 * B, P], F32, tag="zc")
                    nc.scalar.activation(
                        zc[:],
                        gi_t[:, P : 2 * P],
                        mybir.ActivationFunctionType.Sigmoid,
                        scale=-1.0,
                    )
                    n = sml.tile([2 * B, P], F32, tag="n")
                    nc.scalar.activation(
                        n[:],
                        gi_t[:, 2 * P : 3 * P],
                        mybir.ActivationFunctionType.Tanh,
                    )
                    u1 = sml.tile([2 * B, P], F32, tag="u1")
                    nc.vector.tensor_mul(u1[:], zc[:], n[:])
                    hn = hbuf.ap()[:, sl * P : (sl + 1) * P]
                    nc.vector.tensor_add(hn, u1[:], xo_t)
                else:
                    pp = (t - 1) % 8
                    ps = pmm.tile([2 * B, G], F32)
                    for d in (0, 1):
                        for k in range(KB):
                            nc.tensor.matmul(
                                ps[d * B : (d + 1) * B, :],
                                gth_prev[:, (d * NC + k) * B : (d * NC + k + 1) * B],
                                whh_sb.ap()[
                                    :, (d * KB + k) * G : (d * KB + k + 1) * G
                                ],
                                start=(k == 0),
                                stop=(k == KB - 1),
                                tile_position=(0, d * B),
                                skip_group_check=True,
                            )
                    s_rz = srzp.tile([2 * B, 2 * P], F32)
                    nc.vector.tensor_add(s_rz[:], gi_t[:, : 2 * P], ps[:, : 2 * P])
                    rz = rzp.tile([2 * B, 2 * P], F32)
                    nc.scalar.activation(
                        rz[:], s_rz[:], mybir.ActivationFunctionType.Sigmoid
                    )
                    zc = sml.tile([2 * B, P], F32, tag="zc")
                    nc.scalar.activation(
                        zc[:],
                        s_rz[:, P : 2 * P],
                        mybir.ActivationFunctionType.Sigmoid,
                        scale=-1.0,
                    )
                    gn = ps[:, 2 * P : 3 * P]
                    if with_nbias:
                        gnb = sml.tile([2 * B, P], F32, tag="gnb")
                        nc.vector.tensor_add(gnb[:], gn, nbias_sb.ap())
                        gn = gnb[:]
                    t1 = sml.tile([2 * B, P], F32, tag="t1")
                    nc.vector.tensor_mul(t1[:], rz[:, :P], gn)
                    t2 = sml.tile([2 * B, P], F32, tag="t2")
                    nc.vector.tensor_add(t2[:], t1[:], gi_t[:, 2 * P : 3 * P])
                    n = sml.tile([2 * B, P], F32, tag="n")
                    nc.scalar.activation(
                        n[:], t2[:], mybir.ActivationFunctionType.Tanh
                    )
                    zh = sml.tile([2 * B, P], F32, tag="zh")
                    nc.vector.tensor_mul(
                        zh[:], rz[:, P : 2 * P], hbuf.ap()[:, pp * P : (pp + 1) * P]
                    )
                    u1 = sml.tile([2 * B, P], F32, tag="u1")
                    nc.vector.tensor_mul(u1[:], zc[:], n[:])
                    u2 = sml.tile([2 * B, P], F32, tag="u2")
                    nc.vector.tensor_add(u2[:], u1[:], zh[:])
                    hn = hbuf.ap()[:, sl * P : (sl + 1) * P]
                    nc.vector.tensor_add(hn, u2[:], xo_t)

                # flush output rows in 4-step blocks (slot-aligned in the ring)
                if t >= L and (t % 4 == 3 or t == TS - 1):
                    lo = max(t - (t % 4), L)
                    nn_ = t + 1 - lo
                    s0 = lo % 8
                    for d in (0, 1):
                        nc.sync.dma_start(
                            outp[d, lo - L : t + 1 - L].rearrange("s b c -> b s c"),
                            hbuf.ap()[
                                d * B : (d + 1) * B, s0 * P : (s0 + nn_) * P
                            ].rearrange("q (s c) -> q s c", c=P),
                        )

                # --- exchange h.T chunks via AllGather (skip on final step) ---
                if t == TS - 1:
                    continue
                tp = ptr.tile([P, 2 * B], F32)
                nc.tensor.transpose(tp[:], hn, ident_sb.ap())
                snd = sndp.tile([P, 2 * B], F32)
                nc.scalar.copy(snd[:], tp[:])
                if ablate == "noexch":
                    if gth_prev is None:
                        gth = gthp.tile([P, 2 * NC * B], F32)
                        for k in range(2 * NC):
                            nc.vector.tensor_copy(
                                gth[:, k * B : (k + 1) * B], snd[:, :B]
                            )
                        gth_prev = gth
                    continue
                cin = cinp.tile([P, 2 * B], F32)
                nc.sync.dma_start(cin[:], snd[:])
                cout = coutp.tile([NC * P, 2 * B], F32, addr_space="Shared")
                nc.gpsimd.collective_compute(
                    "AllGather",
                    mybir.AluOpType.bypass,
                    replica_groups=[list(range(NC))],
                    ins=[cin.opt()],
                    outs=[cout.opt()],
                )
                # gathered h.T back to SBUF: [128, (d, k, B)] with slot k from
                # rank k's rows [128k:128k+128], cols d*B:(d+1)*B
                gth = gthp.tile([P, 2 * NC * B], F32)
                nc.sync.dma_start(
                    gth[:].rearrange("p (d k j) -> p d k j", d=2, j=B),
                    cout[:].rearrange("(k p) (d j) -> p d k j", p=P, j=B),
                )
                gth_prev = gth
    return []


def patch_deferred_waits(nc, deferred):
    assert not deferred


def make_in_maps(inputs: dict, core: int, shared: dict | None = None) -> dict:
    x = np.asarray(inputs["input_x"], np.float32)[:, :, :F]  # [B, T, F]
    own = slice(core * P, (core + 1) * P)
    if shared is None:
        shared = {}

    def own_cols(w):  # [3F, F] -> W.T own cols [F, 384]
        wt = np.ascontiguousarray(np.asarray(w, np.float32).T)
        return np.concatenate(
            [wt[:, g * F + core * P : g * F + (core + 1) * P] for g in range(3)],
            axis=1,
        )

    def own_vec(v):
        v = np.asarray(v, np.float32)
        return np.concatenate(
            [v[g * F + core * P : g * F + (core + 1) * P] for g in range(3)]
        )

    if "xt" not in shared:
        # x.T in t-major column order; each core ships only its T/8 slice
        shared["xt"] = np.ascontiguousarray(x.transpose(2, 1, 0).reshape(F, T * B))
    TB8 = T * B // NC
    m = {
        "xt": np.ascontiguousarray(shared["xt"][:, core * TB8 : (core + 1) * TB8]),
        "wih": np.ascontiguousarray(
            np.stack(
                [own_cols(inputs["Wih_f"]).reshape(KB, P, G),
                 own_cols(inputs["Wih_b"]).reshape(KB, P, G)]
            )
        ),
        "whh": np.ascontiguousarray(
            np.stack(
                [own_cols(inputs["Whh_f"]).reshape(KB, P, G),
                 own_cols(inputs["Whh_b"]).reshape(KB, P, G)]
            )
        ),
        "ident": np.eye(2 * B, dtype=np.float32),
        "identP": np.eye(P, dtype=np.float32),
    }
    # gate biases: bih (all gates) + bhh (r,z only) fold into gi; bhh_n is
    # applied inside the n-gate (it is multiplied by r together with gh_n).
    gb = []
    nb = []
    for d, (bi, bh) in enumerate(
        [(inputs["bih_f"], inputs["bhh_f"]), (inputs["bih_b"], inputs["bhh_b"])]
    ):
        bio, bho = own_vec(bi), own_vec(bh)
        gv = bio.copy()
        gv[: 2 * P] += bho[: 2 * P]
        gb.append(np.broadcast_to(gv, (P, G)))
        nb.append(np.broadcast_to(bho[2 * P :], (B, P)))
    m["_gbias"] = np.ascontiguousarray(np.stack(gb))  # [2, P, G]
    m["_nbias"] = np.ascontiguousarray(np.concatenate(nb, axis=0))  # [2B, P]
    return m


_COMPILED = {}


def _get_compiled(with_gbias: bool, with_nbias: bool):
    key = (with_gbias, with_nbias, os.environ.get("K_ABLATE", ""))
    if key not in _COMPILED:
        nc = bacc.Bacc(
            "TRN2",
            target_bir_lowering=False,
            debug=False,
            enable_asserts=True,
            num_devices=NC,
        )
        with tile.TileContext(nc) as tc:
            deferred = build_gru_kernel(nc, tc, with_gbias, with_nbias)
        patch_deferred_waits(nc, deferred)
        nc.compile()
        _COMPILED[key] = nc
    return _COMPILED[key]


def kernel(**inputs) -> np.ndarray:
    shared = {}
    maps = [make_in_maps(inputs, c, shared) for c in range(NC)]
    with_gbias = any(np.any(m["_gbias"]) for m in maps)
    with_nbias = any(np.any(m["_nbias"]) for m in maps)
    in_maps = []
    for m in maps:
        gb, nb = m.pop("_gbias"), m.pop("_nbias")
        if with_gbias:
            m["gbias"] = gb
        if with_nbias:
            m["nbias"] = nb
        in_maps.append(m)

    nc = _get_compiled(with_gbias, with_nbias)
    res = bass_utils.run_bass_kernel_spmd(nc, in_maps, core_ids=list(range(NC)))

    TO = T - 2 * L
    out = np.empty((B, TO, 2 * F), np.float32)
    for c in range(NC):
        oo = np.asarray(res.results[c]["out_own"])  # [2, TO, B, P]
        out[:, :, c * P : (c + 1) * P] = oo[0].transpose(1, 0, 2)
        out[:, :, F + c * P : F + (c + 1) * P] = oo[1].transpose(1, 0, 2)
    return out



# revision 7
# speedup vs baseline: 4.6646x; 4.6646x over previous
"""BiGRU encoder (nn_BiGRUEncoder) as an 8-core TRN2 Bass kernel.

Contract: kernel(**inputs) takes the FULL unsharded inputs from
setup_inputs() and returns the FULL [B, T-2L, 2F] output, distributing work
across 8 NeuronCores internally.

Decomposition: the hidden dim F=1024 is split across the 8 cores (128
features each). Every core runs BOTH scan directions with the full batch
B=32, computing its 384 rows of the 3F gate pre-activations per step. After
each step the transposed h chunks ([128, 32] per direction) are exchanged
with an AllGather so the next step's recurrent matmul has the full h.T.
Input projections gi = x @ Wih.T don't depend on h and are hoisted into a
prologue as one large batched matmul per direction, stored in DRAM, and
streamed per step.

Wall time is dominated by the host<->device tunnel (~50 MB/s), so all
host<->device payloads are fp16: x ships rows-major [T*B/8, F] per core
(the transposes the gate matmul needs are done on the device), gate/
recurrent weights ship fp16 (cached on device across calls behind a
content digest), and the output returns fp16. The PJRT wrapper is built
once and cached; the output-donation zero buffers are created on-device
inside the jit instead of being uploaded each call.
"""

import sys

sys.path.insert(0, "/opt/trn_rl_repo")

import hashlib
import os
import time

import numpy as np

from concourse import bass, bacc, tile, mybir
from concourse import bass2jax

F32 = mybir.dt.float32
F16 = mybir.dt.float16

B = 32  # batch
T = 512  # sequence length
F = 1024  # hidden/feature dim
L = 10  # trim at both ends of T
NC = 8  # cores
P = 128  # partitions / features per core
G = 3 * P  # gate rows per core
KB = F // P  # contraction blocks
TB8 = T * B // NC  # rows of (t, b) per core
TO = T - 2 * L  # output steps


def build_gru_kernel(nc, tc, with_gbias: bool, with_nbias: bool):
    """Emit the SPMD program (identical on all 8 cores)."""
    TS = T - L  # scan steps needed

    x8 = nc.dram_tensor("x8", [TB8, F], F16, kind="ExternalInput").ap()
    wih = nc.dram_tensor("wih", [2, KB, P, G], F16, kind="ExternalInput").ap()
    whh = nc.dram_tensor("whh", [2, KB, P, G], F16, kind="ExternalInput").ap()
    ident = nc.dram_tensor("ident", [2 * B, 2 * B], F32, kind="ExternalInput").ap()
    identP = nc.dram_tensor("identP", [P, P], F16, kind="ExternalInput").ap()
    if with_gbias:
        gbias = nc.dram_tensor("gbias", [2, P, G], F32, kind="ExternalInput").ap()
    if with_nbias:
        nbias = nc.dram_tensor("nbias", [2 * B, P], F32, kind="ExternalInput").ap()
    outp = nc.dram_tensor("out_own", [2, B, TO, P], F16, kind="ExternalOutput").ap()

    whh_sb = nc.alloc_sbuf_tensor("whh_sb", [P, 2 * KB * G], F16)
    hbuf = nc.alloc_sbuf_tensor("hbuf", [2 * B, 8 * P], F32)
    ident_sb = nc.alloc_sbuf_tensor("ident_sb", [2 * B, 2 * B], F32)
    identP_sb = nc.alloc_sbuf_tensor("identP_sb", [P, P], F16)
    if with_gbias:
        gbias_sb = nc.alloc_sbuf_tensor("gbias_sb", [P, 2 * G], F32)
    if with_nbias:
        nbias_sb = nc.alloc_sbuf_tensor("nbias_sb", [2 * B, P], F32)

    # ================= prologue =================
    for d in (0, 1):
        for k in range(KB):
            off = (d * KB + k) * G
            nc.sync.dma_start(whh_sb.ap()[:, off : off + G], whh[d, k])
    nc.sync.dma_start(ident_sb.ap(), ident)
    nc.sync.dma_start(identP_sb.ap(), identP)
    if with_gbias:
        for d in (0, 1):
            nc.sync.dma_start(gbias_sb.ap()[:, d * G : (d + 1) * G], gbias[d])
    if with_nbias:
        nc.sync.dma_start(nbias_sb.ap(), nbias)
    nc.vector.memset(hbuf.ap(), 0.0)

    # Bulk input projections, T-sliced: this core computes gi for ALL
    # cores' gate columns over its own T/8 slice of x, then an AllToAll
    # gives every core its own 384 columns for all T. Wih is shipped
    # own-cols and AllGathered to full on device (cuts H2D 8x).
    pidv = nc.sync.partition_id()
    with tc.tile_pool(name="wag", bufs=1, space="DRAM") as wag:
        wihf = [
            wag.tile([NC * KB * P, G], F16, name=f"wihf{d}", addr_space="Shared")
            for d in (0, 1)
        ]
        win = [wag.tile([KB * P, G], F16, name=f"win{d}") for d in (0, 1)]
        for d in (0, 1):
            nc.sync.dma_start(win[d][:], wih[d].rearrange("k p g -> (k p) g"))
            nc.gpsimd.collective_compute(
                "AllGather",
                mybir.AluOpType.bypass,
                replica_groups=[list(range(NC))],
                ins=[win[d].opt()],
                outs=[wihf[d].opt()],
            )
        # wihf[d] rows: (src_core r, k, p) -> Wih_d.T[128k+p, r's 384 cols]
        a2a_in = [
            wag.tile([NC * TB8, G + P], F16, name=f"a2ain{d}") for d in (0, 1)
        ]
        a2a_out = [
            wag.tile([NC * TB8, G + P], F16, name=f"a2aout{d}") for d in (0, 1)
        ]
        n_m = TB8 // P  # 16 m-tiles over this core's T-slice
        with (
            tc.tile_pool(name="xtp", bufs=3) as xtp,
            tc.tile_pool(name="xT", bufs=2) as xTp,
            tc.tile_pool(name="wfp", bufs=1) as wfp,
            tc.tile_pool(name="gps", bufs=4, space="PSUM") as gps,
            tc.tile_pool(name="gis", bufs=4) as gis,
            tc.tile_pool(name="tpp", bufs=2, space="PSUM") as tpp,
        ):
            # full Wih, both directions, SBUF-resident once: cols (d r k g)
            wfull = wfp.tile([P, 2 * NC * KB * G], F16, tag="wfull")
            for d in (0, 1):
                nc.sync.dma_start(
                    wfull[
                        :, d * NC * KB * G : (d + 1) * NC * KB * G
                    ].rearrange("p (r k g) -> p r k g", r=NC, k=KB),
                    wihf[d][:].rearrange("(r k p) g -> p r k g", p=P, k=KB),
                )
            for m in range(n_m):
                xrow = xtp.tile([P, F], F16)
                nc.sync.dma_start(xrow[:], x8[m * P : (m + 1) * P, :])
                # x.T blocks for the gate matmuls
                xT = xTp.tile([P, KB * P], F16, tag="xT")
                for k in range(KB):
                    xps = tpp.tile([P, P], F16)
                    nc.tensor.transpose(
                        xps[:], xrow[:, k * P : (k + 1) * P], identP_sb.ap()
                    )
                    nc.scalar.copy(xT[:, k * P : (k + 1) * P], xps[:])
                # residual x chunks, straight from the row-major tile
                for r in range(NC):
                    for d in (0, 1):
                        nc.sync.dma_start(
                            a2a_in[d][
                                r * TB8 + m * P : r * TB8 + (m + 1) * P, G : G + P
                            ],
                            xrow[:, r * P : (r + 1) * P],
                        )
                for d in (0, 1):
                    for r in range(NC):
                        ps = gps.tile([P, G], F32)
                        for k in range(KB):
                            nc.tensor.matmul(
                                ps[:],
                                xT[:, k * P : (k + 1) * P],
                                wfull[
                                    :,
                                    ((d * NC + r) * KB + k)
                                    * G : ((d * NC + r) * KB + k + 1)
                                    * G,
                                ],
                                start=(k == 0),
                                stop=(k == KB - 1),
                            )
                        gt = gis.tile([P, G], F16)
                        if with_gbias:
                            nc.vector.tensor_add(
                                gt[:], ps[:], gbias_sb.ap()[:, d * G : (d + 1) * G]
                            )
                        else:
                            nc.scalar.copy(gt[:], ps[:])
                        nc.sync.dma_start(
                            a2a_in[d][r * TB8 + m * P : r * TB8 + (m + 1) * P, :G],
                            gt[:],
                        )
        for d in (0, 1):
            nc.gpsimd.collective_compute(
                "AllToAll",
                mybir.AluOpType.bypass,
                replica_groups=[list(range(NC))],
                ins=[a2a_in[d].opt()],
                outs=[a2a_out[d].opt()],
            )
        # after A2A, shard s of a2a_out[d] holds rows for t in
        # [s*T/8, (s+1)*T/8) x B, own 384 cols + own x chunk -> global
        # t-major order, i.e. exactly gid[d].
        gid = a2a_out

        # ================= scan =================
        with (
            tc.tile_pool(name="gip", bufs=6) as gip,
            tc.tile_pool(name="srz", bufs=3) as srzp,
            tc.tile_pool(name="rzp", bufs=3) as rzp,
            tc.tile_pool(name="sml", bufs=3) as sml,
            tc.tile_pool(name="snd", bufs=3) as sndp,
            tc.tile_pool(name="gth", bufs=3) as gthp,
            tc.tile_pool(name="ofl", bufs=2) as oflp,
            tc.tile_pool(name="cin", bufs=3, space="DRAM") as cinp,
            tc.tile_pool(name="cout", bufs=3, space="DRAM") as coutp,
            tc.tile_pool(name="pmm", bufs=3, space="PSUM") as pmm,
            tc.tile_pool(name="ptr", bufs=2, space="PSUM") as ptr,
        ):
            gth_prev = None
            for t in range(TS):
                gi_t = gip.tile([2 * B, G + P], F16)
                nc.sync.dma_start(gi_t[:B, :], gid[0][t * B : (t + 1) * B, :])
                idx = T - 1 - t
                nc.sync.dma_start(gi_t[B:, :], gid[1][idx * B : (idx + 1) * B, :])
                xo_t = gi_t[:, G : G + P]

                sl = t % 8
                if t == 0:
                    # h(-1) = 0 -> gh = 0: h = (1-z)*n + x
                    zc = sml.tile([2 * B, P], F32, tag="zc")
                    nc.scalar.activation(
                        zc[:],
                        gi_t[:, P : 2 * P],
                        mybir.ActivationFunctionType.Sigmoid,
                        scale=-1.0,
                    )
                    n = sml.tile([2 * B, P], F32, tag="n")
                    nc.scalar.activation(
                        n[:],
                        gi_t[:, 2 * P : 3 * P],
                        mybir.ActivationFunctionType.Tanh,
                    )
                    u1 = sml.tile([2 * B, P], F32, tag="u1")
                    nc.vector.tensor_mul(u1[:], zc[:], n[:])
                    hn = hbuf.ap()[:, sl * P : (sl + 1) * P]
                    nc.vector.tensor_add(hn, u1[:], xo_t)
                else:
                    pp = (t - 1) % 8
                    ps = pmm.tile([2 * B, G], F32)
                    for d in (0, 1):
                        for k in range(KB):
                            nc.tensor.matmul(
                                ps[d * B : (d + 1) * B, :],
                                gth_prev[:, (d * NC + k) * B : (d * NC + k + 1) * B],
                                whh_sb.ap()[
                                    :, (d * KB + k) * G : (d * KB + k + 1) * G
                                ],
                                start=(k == 0),
                                stop=(k == KB - 1),
                                tile_position=(0, d * B),
                                skip_group_check=True,
                            )
                    s_rz = srzp.tile([2 * B, 2 * P], F32)
                    nc.vector.tensor_add(s_rz[:], ps[:, : 2 * P], gi_t[:, : 2 * P])
                    rz = rzp.tile([2 * B, 2 * P], F32)
                    nc.scalar.activation(
                        rz[:], s_rz[:], mybir.ActivationFunctionType.Sigmoid
                    )
                    zc = sml.tile([2 * B, P], F32, tag="zc")
                    nc.scalar.activation(
                        zc[:],
                        s_rz[:, P : 2 * P],
                        mybir.ActivationFunctionType.Sigmoid,
                        scale=-1.0,
                    )
                    gn = ps[:, 2 * P : 3 * P]
                    if with_nbias:
                        gnb = sml.tile([2 * B, P], F32, tag="gnb")
                        nc.vector.tensor_add(gnb[:], gn, nbias_sb.ap())
                        gn = gnb[:]
                    t1 = sml.tile([2 * B, P], F32, tag="t1")
                    nc.vector.tensor_mul(t1[:], rz[:, :P], gn)
                    t2 = sml.tile([2 * B, P], F32, tag="t2")
                    nc.vector.tensor_add(t2[:], t1[:], gi_t[:, 2 * P : 3 * P])
                    n = sml.tile([2 * B, P], F32, tag="n")
                    nc.scalar.activation(
                        n[:], t2[:], mybir.ActivationFunctionType.Tanh
                    )
                    zh = sml.tile([2 * B, P], F32, tag="zh")
                    nc.vector.tensor_mul(
                        zh[:], rz[:, P : 2 * P], hbuf.ap()[:, pp * P : (pp + 1) * P]
                    )
                    u1 = sml.tile([2 * B, P], F32, tag="u1")
                    nc.vector.tensor_mul(u1[:], zc[:], n[:])
                    u2 = sml.tile([2 * B, P], F32, tag="u2")
                    nc.vector.tensor_add(u2[:], u1[:], zh[:])
                    hn = hbuf.ap()[:, sl * P : (sl + 1) * P]
                    nc.vector.tensor_add(hn, u2[:], xo_t)

                # flush output rows in 4-step blocks (slot-aligned in the ring)
                if t >= L and (t % 4 == 3 or t == TS - 1):
                    lo = max(t - (t % 4), L)
                    nn_ = t + 1 - lo
                    s0 = lo % 8
                    of = oflp.tile([2 * B, 4 * P], F16)
                    nc.scalar.copy(
                        of[:, : nn_ * P], hbuf.ap()[:, s0 * P : (s0 + nn_) * P]
                    )
                    for d in (0, 1):
                        nc.sync.dma_start(
                            outp[d, :, lo - L : t + 1 - L, :],
                            of[d * B : (d + 1) * B, : nn_ * P].rearrange(
                                "q (s c) -> q s c", c=P
                            ),
                        )

                # --- exchange h.T chunks via AllGather (skip on final step) ---
                if t == TS - 1:
                    continue
                tp = ptr.tile([P, 2 * B], F32)
                nc.tensor.transpose(tp[:], hn, ident_sb.ap())
                snd = sndp.tile([P, 2 * B], F16)
                nc.scalar.copy(snd[:], tp[:])
                cin = cinp.tile([P, 2 * B], F16)
                nc.sync.dma_start(cin[:], snd[:])
                cout = coutp.tile([NC * P, 2 * B], F16, addr_space="Shared")
                nc.gpsimd.collective_compute(
                    "AllGather",
                    mybir.AluOpType.bypass,
                    replica_groups=[list(range(NC))],
                    ins=[cin.opt()],
                    outs=[cout.opt()],
                )
                # gathered h.T back to SBUF: [128, (d, k, B)] with slot k from
                # rank k's rows [128k:128k+128], cols d*B:(d+1)*B
                gth = gthp.tile([P, 2 * NC * B], F16)
                nc.sync.dma_start(
                    gth[:].rearrange("p (d k j) -> p d k j", d=2, j=B),
                    cout[:].rearrange("(k p) (d j) -> p d k j", p=P, j=B),
                )
                gth_prev = gth
    return []


_COMPILED = {}


def _get_compiled(with_gbias: bool, with_nbias: bool):
    key = (with_gbias, with_nbias)
    if key not in _COMPILED:
        nc = bacc.Bacc(
            "TRN2",
            target_bir_lowering=False,
            debug=False,
            enable_asserts=True,
            num_devices=NC,
        )
        with tile.TileContext(nc) as tc:
            build_gru_kernel(nc, tc, with_gbias, with_nbias)
        nc.compile()
        _COMPILED[key] = nc
    return _COMPILED[key]


# ---------------- host side ----------------


def _own_cols(w, core):
    """[3F, F] -> W.T own cols [F, 384] fp16."""
    wt = np.asarray(w, np.float32).T
    return np.concatenate(
        [
            wt[:, g * F + core * P : g * F + (core + 1) * P]
            for g in range(3)
        ],
        axis=1,
    ).astype(np.float16)


def _own_vec(v, core):
    v = np.asarray(v, np.float32)
    return np.concatenate(
        [v[g * F + core * P : g * F + (core + 1) * P] for g in range(3)]
    )


def _prep_weights(inputs):
    """Global (concat over cores along axis 0) weight/bias arrays."""
    wih_g = np.empty((NC * 2, KB, P, G), np.float16)
    whh_g = np.empty((NC * 2, KB, P, G), np.float16)
    for c in range(NC):
        wih_g[2 * c + 0] = _own_cols(inputs["Wih_f"], c).reshape(KB, P, G)
        wih_g[2 * c + 1] = _own_cols(inputs["Wih_b"], c).reshape(KB, P, G)
        whh_g[2 * c + 0] = _own_cols(inputs["Whh_f"], c).reshape(KB, P, G)
        whh_g[2 * c + 1] = _own_cols(inputs["Whh_b"], c).reshape(KB, P, G)
    ident_g = np.tile(np.eye(2 * B, dtype=np.float32), (NC, 1))
    identP_g = np.tile(np.eye(P, dtype=np.float16), (NC, 1))

    # gate biases: bih (all gates) + bhh (r,z only) fold into gi; bhh_n is
    # applied inside the n-gate (it is multiplied by r together with gh_n).
    gb = np.empty((NC * 2, P, G), np.float32)
    nb = np.empty((NC * 2 * B, P), np.float32)
    for c in range(NC):
        for d, (bi, bh) in enumerate(
            [
                (inputs["bih_f"], inputs["bhh_f"]),
                (inputs["bih_b"], inputs["bhh_b"]),
            ]
        ):
            bio, bho = _own_vec(bi, c), _own_vec(bh, c)
            gv = bio.copy()
            gv[: 2 * P] += bho[: 2 * P]
            gb[2 * c + d] = np.broadcast_to(gv, (P, G))
            nb[c * 2 * B + d * B : c * 2 * B + (d + 1) * B] = np.broadcast_to(
                bho[2 * P :], (B, P)
            )
    return {
        "wih": wih_g,
        "whh": whh_g,
        "ident": ident_g,
        "identP": identP_g,
        "_gbias": gb,
        "_nbias": nb,
    }


_W_KEYS = ("Wih_f", "Whh_f", "bih_f", "bhh_f", "Wih_b", "Whh_b", "bih_b", "bhh_b")


def _weights_digest(inputs):
    h = hashlib.blake2b(digest_size=16)
    for k in _W_KEYS:
        a = np.ascontiguousarray(np.asarray(inputs[k]))
        h.update(k.encode())
        h.update(str(a.shape).encode())
        h.update(a.view(np.uint8).data)
    return h.digest()


_RUNNERS = {}


def _get_runner(nc):
    """Build (once) a jitted PJRT wrapper around the compiled Bass module.

    Differences vs bass_utils.run_bass_kernel_spmd under axon: the jit is
    cached across calls (no per-call retrace), and the output-donation zero
    buffers are created on-device inside the jit instead of being uploaded
    through the ~50 MB/s tunnel on every call.
    """
    key = id(nc)
    if key in _RUNNERS:
        return _RUNNERS[key]

    import jax
    from jax.experimental.shard_map import shard_map
    from jax.sharding import Mesh, NamedSharding, PartitionSpec

    bass2jax.install_neuronx_cc_hook()
    assert nc.dbg_addr is None, "dbg callbacks unsupported in cached runner"

    in_names = []
    out_names = []
    out_avals = []
    partition_name = (
        nc.partition_id_tensor.name if nc.partition_id_tensor else None
    )
    for alloc in nc.m.functions[0].allocations:
        if not isinstance(alloc, mybir.MemoryLocationSet):
            continue
        name = alloc.memorylocations[0].name
        if alloc.kind == "ExternalInput":
            if name != partition_name:
                in_names.append(name)
        elif alloc.kind == "ExternalOutput":
            out_names.append(name)
            out_avals.append(
                jax.core.ShapedArray(tuple(alloc.tensor_shape), mybir.dt.np(alloc.dtype))
            )

    # NOTE: unlike run_bass_via_pjrt we do NOT pass zero-filled output
    # buffers: they are only read when a kernel leaves output elements
    # unwritten (the donation trick pre-zeros them), and this kernel writes
    # every element of out_own. Omitting them avoids uploading
    # output-sized zero arrays through the ~50 MB/s tunnel on every call.
    bind_names = list(in_names)
    if partition_name is not None:
        bind_names.append(partition_name)

    def _body(*args):
        ops = list(args)
        if partition_name is not None:
            ops.append(bass2jax.partition_id_tensor())
        outs = bass2jax._bass_exec_p.bind(
            *ops,
            out_avals=tuple(out_avals),
            in_names=tuple(bind_names),
            out_names=tuple(out_names),
            lowering_input_output_aliases=(),
            sim_require_finite=True,
            sim_require_nnan=True,
            nc=nc,
        )
        return tuple(outs)

    devices = jax.devices()[:NC]
    mesh = Mesh(np.asarray(devices), ("core",))
    spec = PartitionSpec("core")
    fn = jax.jit(
        shard_map(
            _body,
            mesh=mesh,
            in_specs=(spec,) * len(in_names),
            out_specs=(spec,) * len(out_names),
            check_rep=False,
        ),
        keep_unused=True,
    )
    sharding = NamedSharding(mesh, spec)
    _RUNNERS[key] = (fn, in_names, out_names, sharding)
    return _RUNNERS[key]


_WCACHE = {"digest": None, "dev": None, "flags": None}


def kernel(**inputs) -> np.ndarray:
    import jax

    timing = bool(os.environ.get("K_TIME"))
    t0 = time.time()

    # ---- constant (weight) inputs: device-cached behind a digest ----
    digest = _weights_digest(inputs)
    if _WCACHE["digest"] != digest:
        wmap = _prep_weights(inputs)
        gb, nb = wmap.pop("_gbias"), wmap.pop("_nbias")
        with_gbias = bool(np.any(gb))
        with_nbias = bool(np.any(nb))
        if with_gbias:
            wmap["gbias"] = gb
        if with_nbias:
            wmap["nbias"] = nb
        nc = _get_compiled(with_gbias, with_nbias)
        fn, in_names, out_names, sharding = _get_runner(nc)
        dev = {
            k: jax.device_put(v, sharding) for k, v in wmap.items()
        }
        _WCACHE.update(
            digest=digest, dev=dev, flags=(with_gbias, with_nbias)
        )
    else:
        nc = _get_compiled(*_WCACHE["flags"])
        fn, in_names, out_names, sharding = _get_runner(nc)
        dev = _WCACHE["dev"]
    t1 = time.time()

    # ---- per-call input: x, rows-major (t, b) fp16 ----
    x = np.asarray(inputs["input_x"])
    x8 = np.ascontiguousarray(
        x.transpose(1, 0, 2)[:, :, :F].astype(np.float16).reshape(T * B, F)
    )
    t2 = time.time()

    arrs = {"x8": x8, **dev}
    outs = fn(*[arrs[n] for n in in_names])
    og = np.asarray(outs[out_names.index("out_own")]).reshape(NC, 2, B, TO, P)
    t3 = time.time()

    out = np.empty((B, TO, 2 * F), np.float32)
    for c in range(NC):
        out[:, :, c * P : (c + 1) * P] = og[c, 0]
        out[:, :, F + c * P : F + (c + 1) * P] = og[c, 1]
    t4 = time.time()
    if timing:
        print(
            f"[kernel] weights {t1 - t0:.3f}s  x-prep {t2 - t1:.3f}s  "
            f"run+fetch {t3 - t2:.3f}s  asm {t4 - t3:.3f}s  total {t4 - t0:.3f}s"
        )
    return out
